# revision 5
# baseline (speedup 1.0000x reference)
"""Trainium2 Bass kernel for DecouplePreAggGraphConv (GNN message passing).

out[b,j,:] = diag(adj)[j] * (x[b,j] @ W0[j])
           + sum_k offdiag(adj)[j,k] * (x[b,k] @ W1[k])
           + bias

Data-parallel over B across 8 NeuronCores. Per core, per 128-row batch
tile:
  1. one DMA load of x-tile [128, J*128]
  2. PE transposes per joint -> xT_k [n, b] (via identity matmul)
  3. per-joint GEMM  h_k = xT_k.T @ [diag_k*W0_k | W1_k]  -> PSUM [128,256]
  4. drain h to SBUF, then SBUF->SBUF DMA reshuffle into a
     (3-batch-row-group, 35-row) layout: rows = [17 h1 | 17 h0s | bias]
  5. mixing GEMM with a constant block-diagonal [105,51] stationary
     matrix (off.T / I / ones blocks) computes the adjacency mix, the
     self term and the bias add in one pass -> PSUM [51, (g,m)]
  6. drain + one strided store straight into out[b,j,m] layout.
"""

import sys

sys.path.insert(0, "/opt/trn_rl_repo")

import numpy as np

import concourse.bass as bass
import concourse.mybir as mybir
import concourse.tile as tile
from concourse import bacc
from concourse.bass_utils import run_bass_kernel_spmd

B, J, FIN, FOUT = 16384, 17, 128, 128
N_CORES = 8
TB = 128            # batch rows per tile
CJ = J * FOUT       # 2176
G3 = TB // 3        # 42 full groups of 3 rows (126), 2-row tail
MAIN = 3 * G3       # 126
HPF = G3 * FOUT     # 5376 free size of the reshuffled tile
MIXCH = 1024        # mix psum chunk (free elems)
F32 = mybir.dt.float32

_prog_cache: dict[int, object] = {}


def _build_program(bs: int):
    """Build the SPMD Bass program for a per-core batch shard of `bs` rows."""
    nt = bs // TB
    assert bs % TB == 0

    nc = bacc.Bacc("TRN2", target_bir_lowering=False, debug=False,
                   num_devices=N_CORES)

    xs = nc.declare_dram_parameter("xs", [bs, J, FIN], F32, isOutput=False)
    wcat = nc.declare_dram_parameter("wcat", [FIN, J, 2 * FOUT], F32,
                                     isOutput=False)
    mix3 = nc.declare_dram_parameter("mix3", [105, 51], F32, isOutput=False)
    mix2 = nc.declare_dram_parameter("mix2", [70, 34], F32, isOutput=False)
    bias3 = nc.declare_dram_parameter("bias3", [3, HPF], F32, isOutput=False)
    ident = nc.declare_dram_parameter("ident", [128, 128], F32, isOutput=False)
    out = nc.declare_dram_parameter("out", [bs, J, FOUT], F32, isOutput=True)

    with tile.TileContext(nc) as tc:
        with (
            tc.tile_pool(name="const", bufs=1) as cpool,
            tc.tile_pool(name="x", bufs=2) as xpool,
            tc.tile_pool(name="xt", bufs=3) as xtpool,
            tc.tile_pool(name="hsb", bufs=2) as hpool,
            tc.tile_pool(name="hp", bufs=2) as hppool,
            tc.tile_pool(name="osb", bufs=2) as opool,
            tc.tile_pool(name="tp", bufs=2, space=bass.MemorySpace.PSUM) as tpp,
            tc.tile_pool(name="hps", bufs=2, space=bass.MemorySpace.PSUM) as hpsp,
            tc.tile_pool(name="mxp", bufs=2, space=bass.MemorySpace.PSUM) as mxpp,
        ):
            # ---- constants, loaded once ----
            wcat_sb = cpool.tile([FIN, J, 2 * FOUT], F32, tag="wcat")
            nc.sync.dma_start(wcat_sb[:], wcat[:])
            mix3_sb = cpool.tile([105, 51], F32, tag="mix3")
            nc.sync.dma_start(mix3_sb[:], mix3[:])
            mix2_sb = cpool.tile([70, 34], F32, tag="mix2")
            nc.sync.dma_start(mix2_sb[:], mix2[:])
            id_sb = cpool.tile([128, 128], F32, tag="ident")
            nc.sync.dma_start(id_sb[:], ident[:])

            for t in range(nt):
                b0 = t * TB
                # 1. load x tile
                x_t = xpool.tile([TB, J, FIN], F32, tag="x")
                nc.sync.dma_start(x_t[:], xs[b0:b0 + TB])

                # 2/3/4a. per joint: transpose, GEMM, drain
                h_sb = hpool.tile([TB, J, 2 * FOUT], F32, tag="h")
                for k in range(J):
                    tp = tpp.tile([128, TB], F32, tag="tp")
                    nc.tensor.transpose(tp[:], x_t[:, k, :], id_sb[:])
                    xt = xtpool.tile([128, TB], F32, tag="xt")
                    if k % 2 == 0:
                        nc.vector.tensor_copy(xt[:], tp[:])
                    else:
                        nc.scalar.copy(xt[:], tp[:])
                    hk = hpsp.tile([TB, 2 * FOUT], F32, tag="hk")
                    nc.tensor.matmul(hk[:], xt[:], wcat_sb[:, k, :])
                    if k % 2 == 0:
                        nc.scalar.copy(h_sb[:, k, :], hk[:])
                    else:
                        nc.vector.tensor_copy(h_sb[:, k, :], hk[:])

                # 4b. reshuffle into group layout
                # rows r in [0,17): h1_k ; [17,34): h0s_k ; 34: bias
                # (SBUF APs need partition as dim 0 on both sides, so this
                #  is one DMA per (joint, group-phase, half) - 102 total,
                #  split across the two HWDGE rings.)
                hp_t = hppool.tile([105, HPF], F32, tag="hp")
                hp_tail = hppool.tile([70, FOUT], F32, tag="hptail")
                for k in range(J):
                    dq = nc.sync if k % 2 == 0 else nc.scalar
                    for i in range(3):
                        src = h_sb[i:MAIN:3, k, :]
                        r1 = i * 35 + k
                        r0 = i * 35 + 17 + k
                        dq.dma_start(hp_t[r1:r1 + 1, :], src[:, FOUT:])
                        dq.dma_start(hp_t[r0:r0 + 1, :], src[:, :FOUT])
                nc.sync.dma_start(hp_t[34:105:35, :], bias3[:])
                for i in range(2):
                    r = MAIN + i
                    nc.scalar.dma_start(hp_tail[i * 35:i * 35 + 17, :],
                                        h_sb[r:r + 1, :, FOUT:])
                    nc.scalar.dma_start(hp_tail[i * 35 + 17:i * 35 + 34, :],
                                        h_sb[r:r + 1, :, :FOUT])
                nc.sync.dma_start(hp_tail[34:70:35, :], bias3[0:2, 0:FOUT])

                # 5/6. mix GEMM chunks, drain, store
                o_sb = opool.tile([51, HPF], F32, tag="osb")
                nch = (HPF + MIXCH - 1) // MIXCH
                for c in range(nch):
                    f0 = c * MIXCH
                    fw = min(MIXCH, HPF - f0)
                    mp = mxpp.tile([51, MIXCH], F32, tag="mx")
                    for s0 in range(0, fw, 512):
                        sw = min(512, fw - s0)
                        nc.tensor.matmul(mp[:, s0:s0 + sw], mix3_sb[:],
                                         hp_t[:, f0 + s0:f0 + s0 + sw])
                    eng = nc.vector if c % 2 == 0 else nc.scalar
                    if eng is nc.vector:
                        eng.tensor_copy(o_sb[:, f0:f0 + fw], mp[:, :fw])
                    else:
                        eng.copy(o_sb[:, f0:f0 + fw], mp[:, :fw])

                ot_sb = opool.tile([34, FOUT], F32, tag="otail")
                mpt = mxpp.tile([34, FOUT], F32, tag="mx")
                nc.tensor.matmul(mpt[:], mix2_sb[:], hp_tail[:])
                nc.vector.tensor_copy(ot_sb[:], mpt[:])

                dst = out[b0:b0 + MAIN].rearrange("(g i) j m -> i j g m", i=3)
                nc.sync.dma_start(dst, o_sb[:])
                nc.sync.dma_start(out[b0 + MAIN:b0 + TB], ot_sb[:])

    nc.compile()
    return nc


def _host_prep(x, W, bias, adj, bs):
    """Build the per-core input maps."""
    diag = np.diagonal(adj).astype(np.float32)
    off = (adj * (1.0 - np.eye(J, dtype=adj.dtype))).astype(np.float32)

    # stage-1 weights: [FIN, J, 2*FOUT], columns = [diag_k*W0_k | W1_k]
    wcat = np.concatenate([diag[:, None, None] * W[0], W[1]], axis=2)
    wcat = np.ascontiguousarray(wcat.transpose(1, 0, 2)).astype(np.float32)

    # mixing stationary block: [35, 17] per 3-row group
    mixblock = np.zeros((35, J), dtype=np.float32)
    mixblock[0:J, :] = off.T          # h1 rows: sum_k off[j,k] h1_k
    mixblock[J:2 * J, :] = np.eye(J, dtype=np.float32)  # h0s rows
    mixblock[2 * J, :] = 1.0          # bias row
    z = np.zeros_like(mixblock)
    mix3 = np.block([[mixblock, z, z], [z, mixblock, z], [z, z, mixblock]])
    mix2 = np.block([[mixblock, z], [z, mixblock]])

    bias3 = np.tile(bias.astype(np.float32), (3, G3))
    ident = np.eye(128, dtype=np.float32)

    shared = {
        "wcat": wcat,
        "mix3": np.ascontiguousarray(mix3),
        "mix2": np.ascontiguousarray(mix2),
        "bias3": np.ascontiguousarray(bias3),
        "ident": ident,
    }
    in_maps = []
    for c in range(N_CORES):
        m = dict(shared)
        m["xs"] = np.ascontiguousarray(x[c * bs:(c + 1) * bs])
        in_maps.append(m)
    return in_maps


def _run(x, W, bias, adj, bs, profile=False, tmpdir=None):
    if bs not in _prog_cache:
        _prog_cache[bs] = _build_program(bs)
    nc = _prog_cache[bs]
    in_maps = _host_prep(x, W, bias, adj, bs)
    res = run_bass_kernel_spmd(nc, in_maps, list(range(N_CORES)),
                               trace=profile, tmpdir=tmpdir)
    out = np.concatenate([res.results[c]["out"] for c in range(N_CORES)],
                         axis=0)
    if profile:
        return out, res
    return out


def kernel(x, W, bias, adj):
    x = np.asarray(x, dtype=np.float32)
    W = np.asarray(W, dtype=np.float32)
    bias = np.asarray(bias, dtype=np.float32)
    adj = np.asarray(adj, dtype=np.float32)
    assert x.shape == (B, J, FIN)
    return _run(x, W, bias, adj, B // N_CORES)
